# revision 1
# baseline (speedup 1.0000x reference)
"""Trainium2 Bass kernel for nn_Decoder (2-layer LSTM + 3 FC + top-k beam decode).

Strategy: pure data parallelism over batch (2048 -> 8 cores x 256).
All activations feature-major [feat, batch]. LSTM/fc1/fc2 in fp32 on PE;
fc3 (256->10000) as a 4-term bf16-split matmul (fp32 accuracy, smaller SBUF).
Argmax/top-k via DVE max8 + max_index; embedding gather via one-hot matmul.
Host assembles the [B,16,4,2] trajectory from per-step argmax indices.
"""
import numpy as np
import ml_dtypes

B, D, H = 2048, 256, 256
K4, QW, QL, DELTA = 4, 100, 100, 16
Q = QW * QL
NCORES = 8
BS = B // NCORES          # 256 rows per core
NT = 20                   # fc3 N-tiles of 500
TW = 500

_CACHE = {}


def _build_nc(delta=DELTA, dbg=False):
    import concourse.mybir as mybir
    import concourse.tile as tile
    import concourse.bacc as bacc
    from concourse.masks import make_identity

    F32 = mybir.dt.float32
    BF16 = mybir.dt.bfloat16
    U32 = mybir.dt.uint32
    I32 = mybir.dt.int32
    AF = mybir.ActivationFunctionType
    ALU = mybir.AluOpType

    nc = bacc.Bacc(None, target_bir_lowering=False, debug=False)

    def din(name, shape, dt=F32):
        return nc.dram_tensor(name, shape, dt, kind="ExternalInput")

    x_in = din("x_fm", [2, 128, BS])
    h1_in = din("h1_fm", [2, 128, BS])
    c1_in = din("c1_fm", [2, 128, BS])
    h2_in = din("h2_fm", [2, 128, BS])
    c2_in = din("c2_fm", [2, 128, BS])
    w1ih_in = din("w1ihT", [2, 128, 4 * H])
    w1hh_in = din("w1hhT", [2, 128, 4 * H])
    w2ih_in = din("w2ihT", [2, 128, 4 * H])
    w2hh_in = din("w2hhT", [2, 128, 4 * H])
    fc1_in = din("fc1T", [2, 128, H])
    fc2_in = din("fc2T", [2, 128, H])
    fc3h_in = din("fc3Th", [2, 128, Q], BF16)
    fc3l_in = din("fc3Tl", [2, 128, Q], BF16)
    fcqw_in = din("fcqwT", [100, 128])
    fcql_in = din("fcqlT", [100, 128])
    b1_in = din("b1r", [128, 8])
    b2_in = din("b2r", [128, 8])
    fc1b_in = din("fc1br", [128, 2])
    fc2b_in = din("fc2br", [128, 2])
    b3rep_in = din("b3rep", [128, Q])
    fcqwb_in = din("fcqwb", [128, 1])
    fcqlb_in = din("fcqlb", [128, 1])

    idx_out = nc.dram_tensor("idx_out", [2, 128, 20], U32, kind="ExternalOutput")
    if dbg:
        dbg_log = nc.dram_tensor("dbg_log", [128, Q], mybir.dt.float32,
                                 kind="ExternalOutput")
        dbg_h1 = nc.dram_tensor("dbg_h1", [128, 2, BS], mybir.dt.float32,
                                kind="ExternalOutput")
        dbg_y2 = nc.dram_tensor("dbg_y2", [128, 2, BS], mybir.dt.float32,
                                kind="ExternalOutput")
        dbg_x = nc.dram_tensor("dbg_x", [128, 2, BS], mybir.dt.float32,
                               kind="ExternalOutput")
        dbg_h1in = nc.dram_tensor("dbg_h1in", [128, 2, BS], mybir.dt.float32,
                                  kind="ExternalOutput")
        dbg_si = nc.dram_tensor("dbg_si", [128, 256], mybir.dt.float32,
                                kind="ExternalOutput")
        dbg_gp = nc.dram_tensor("dbg_gp", [128, 256], mybir.dt.float32,
                                kind="ExternalOutput")

    with tile.TileContext(nc) as tc:
        with (
            tc.tile_pool(name="wp", bufs=1) as wp,
            tc.tile_pool(name="st", bufs=1) as st,
            tc.tile_pool(name="wk", bufs=2) as wk,
            tc.tile_pool(name="ps", bufs=2, space="PSUM") as ps,
        ):
            # ---- load weights / consts ----
            def wload(src, shape, tag, dt=F32):
                t = wp.tile(shape, dt, tag=tag, name=tag)
                if len(shape) == 3 and shape[1] == 2:
                    nc.sync.dma_start(t[:], src[:].rearrange("c p f -> p c f"))
                else:
                    nc.sync.dma_start(t[:], src[:])
                return t

            w1ih = wload(w1ih_in, [128, 2, 4 * H], "w1ih")
            w1hh = wload(w1hh_in, [128, 2, 4 * H], "w1hh")
            w2ih = wload(w2ih_in, [128, 2, 4 * H], "w2ih")
            w2hh = wload(w2hh_in, [128, 2, 4 * H], "w2hh")
            fc1 = wload(fc1_in, [128, 2, H], "fc1")
            fc2 = wload(fc2_in, [128, 2, H], "fc2")
            fc3h = wload(fc3h_in, [128, 2, Q], "fc3h", BF16)
            fc3l = wload(fc3l_in, [128, 2, Q], "fc3l", BF16)
            fcqw = wload(fcqw_in, [100, 128], "fcqw")
            fcql = wload(fcql_in, [100, 128], "fcql")
            b1r = wload(b1_in, [128, 8], "b1r")
            b2r = wload(b2_in, [128, 8], "b2r")
            fc1b = wload(fc1b_in, [128, 2], "fc1b")
            fc2b = wload(fc2b_in, [128, 2], "fc2b")
            b3rep = wload(b3rep_in, [128, Q], "b3rep")
            fcqwb = wload(fcqwb_in, [128, 1], "fcqwb")
            fcqlb = wload(fcqlb_in, [128, 1], "fcqlb")

            ident = wp.tile([128, 128], F32)
            make_identity(nc, ident[:])
            io_f = wp.tile([128, 100], F32)
            nc.gpsimd.iota(io_f[:], pattern=[[1, 100]], base=0, channel_multiplier=0,
                           allow_small_or_imprecise_dtypes=True)
            io100 = wp.tile([128, 100], F32)
            nc.gpsimd.iota(io100[:], pattern=[[100, 100]], base=0,
                           channel_multiplier=0,
                           allow_small_or_imprecise_dtypes=True)
            io32 = wp.tile([128, 32], F32)
            nc.gpsimd.iota(io32[:], pattern=[[1, 32]], base=0, channel_multiplier=0,
                           allow_small_or_imprecise_dtypes=True)

            # ---- persistent states (feature-major [128, chunk, BS]) ----
            def sload(src, tag):
                t = st.tile([128, 2, BS], F32, tag=tag, name=tag)
                nc.sync.dma_start(t[:], src[:].rearrange("c p b -> p c b"))
                return t

            h1_t = sload(h1_in, "h1")
            c1_t = sload(c1_in, "c1")
            h2_t = sload(h2_in, "h2")
            c2_t = sload(c2_in, "c2")
            emb_t = st.tile([128, 2, BS], F32, tag="emb", name="emb")
            nc.sync.dma_start(emb_t[:], x_in[:].rearrange("c p b -> p c b"))
            outi = st.tile([128, 2, 20], U32, tag="outi", name="outi")
            nc.vector.memset(outi[:], 0)



            def pbig():
                return ps.tile([128, 4, 512], F32, tag="big", name="big")

            def lstm_layer(inp, hT, cT, wih, whh, br):
                gpt = pbig()

                def gsl(g):
                    return gpt[:, g // 2, (g % 2) * 256:(g % 2) * 256 + 256]

                for g in range(8):
                    sl = slice(128 * g, 128 * (g + 1))
                    nc.tensor.matmul(gsl(g), wih[:, 0, sl], inp[:, 0, :],
                                     start=True, stop=False)
                    nc.tensor.matmul(gsl(g), wih[:, 1, sl], inp[:, 1, :],
                                     start=False, stop=False)
                    nc.tensor.matmul(gsl(g), whh[:, 0, sl], hT[:, 0, :],
                                     start=False, stop=False)
                    nc.tensor.matmul(gsl(g), whh[:, 1, sl], hT[:, 1, :],
                                     start=False, stop=True)
                for ch in range(2):
                    si = wk.tile([128, 256], F32, tag="si", bufs=1)
                    sf = wk.tile([128, 256], F32, tag="sf", bufs=1)
                    tg = wk.tile([128, 256], F32, tag="tg", bufs=1)
                    so = wk.tile([128, 256], F32, tag="so", bufs=1)
                    if dbg and ch == 0 and wih is w1ih:
                        rawg = wk.tile([128, 256], F32, tag="rawg", name="rawg")
                        nc.scalar.copy(rawg[:], gsl(0))
                        nc.sync.dma_start(dbg_gp[:], rawg[:])
                    nc.scalar.activation(si[:], gsl(0 + ch), AF.Sigmoid,
                                         bias=br[:, 0 + ch:1 + ch])
                    if dbg and ch == 0 and wih is w1ih:
                        nc.sync.dma_start(dbg_si[:], si[:])
                    nc.scalar.activation(sf[:], gsl(2 + ch), AF.Sigmoid,
                                         bias=br[:, 2 + ch:3 + ch])
                    nc.scalar.activation(tg[:], gsl(4 + ch), AF.Tanh,
                                         bias=br[:, 4 + ch:5 + ch])
                    nc.scalar.activation(so[:], gsl(6 + ch), AF.Sigmoid,
                                         bias=br[:, 6 + ch:7 + ch])
                    t1 = wk.tile([128, 256], F32, tag="t1", bufs=1)
                    t2 = wk.tile([128, 256], F32, tag="t2", bufs=1)
                    nc.vector.tensor_mul(t1[:], sf[:], cT[:, ch, :])
                    nc.vector.tensor_mul(t2[:], si[:], tg[:])
                    nc.vector.tensor_add(cT[:, ch, :], t1[:], t2[:])
                    t3 = wk.tile([128, 256], F32, tag="t3", bufs=1)
                    nc.scalar.activation(t3[:], cT[:, ch, :], AF.Tanh)
                    nc.vector.tensor_mul(hT[:, ch, :], so[:], t3[:])

            if dbg:
                nc.sync.dma_start(dbg_x[:], emb_t[:])
                nc.sync.dma_start(dbg_h1in[:], h1_t[:])
            for t in range(delta):
                inp = emb_t
                lstm_layer(inp, h1_t, c1_t, w1ih, w1hh, b1r)
                lstm_layer(h1_t, h2_t, c2_t, w2ih, w2hh, b2r)

                if dbg and t == 0:
                    nc.sync.dma_start(dbg_h1[:], h1_t[:])
                # fc1, fc2 (feature-major out)
                y1 = st.tile([128, 2, BS], F32, tag="y1")
                y2 = st.tile([128, 2, BS], F32, tag="y2")
                for (dst, w, bb, src) in ((y1, fc1, fc1b, h2_t), (y2, fc2, fc2b, y1)):
                    fpt = pbig()
                    for m in range(2):
                        fsl = fpt[:, m // 2, (m % 2) * 256:(m % 2) * 256 + 256]
                        sl = slice(128 * m, 128 * (m + 1))
                        nc.tensor.matmul(fsl, w[:, 0, sl], src[:, 0, :],
                                         start=True, stop=False)
                        nc.tensor.matmul(fsl, w[:, 1, sl], src[:, 1, :],
                                         start=False, stop=True)
                        nc.scalar.activation(dst[:, m, :], fsl, AF.Identity,
                                             bias=bb[:, m:m + 1])

                if dbg and t == 0:
                    nc.sync.dma_start(dbg_y2[:], y2[:])
                # split y2 -> bf16 hi/lo
                y2h = st.tile([128, 2, BS], BF16, tag="y2h")
                y2l = st.tile([128, 2, BS], BF16, tag="y2l")
                nc.vector.tensor_copy(y2h[:], y2[:])
                nc.vector.tensor_sub(y2l[:], y2[:], y2h[:])

                # fc3 per batch-chunk: 5 groups x 4 tiles of 500
                ohwT = wk.tile([100, 256], F32, tag="ohwT", name="ohwT")
                ohlT = wk.tile([100, 256], F32, tag="ohlT", name="ohlT")
                for bc in range(2):
                    bsl = slice(128 * bc, 128 * (bc + 1))
                    lq = [wk.tile([128, 2500], F32, tag="logq", name="logq",
                                  bufs=2) for _ in range(4)]
                    cand_v = wk.tile([128, 32], F32, tag="candv", name="candv")
                    cand_i = wk.tile([128, 32], F32, tag="candi", name="candi")
                    nscan = [0]

                    def scan_ready(upto):
                        # scan any quarter fully evacuated below `upto`
                        while nscan[0] < 4 and (nscan[0] + 1) * 2500 <= upto:
                            qt = nscan[0]
                            m8q = wk.tile([128, 8], F32, tag="m8q", name="m8q")
                            i8q = wk.tile([128, 8], U32, tag="i8q", name="i8q")
                            nc.vector.max(m8q[:], lq[qt][:])
                            nc.vector.max_index(i8q[:], m8q[:], lq[qt][:])
                            nc.vector.tensor_copy(cand_v[:, 8 * qt:8 * qt + 8],
                                                  m8q[:])
                            i8f = wk.tile([128, 8], F32, tag="i8f", name="i8f")
                            nc.vector.tensor_copy(i8f[:], i8q[:])
                            nc.vector.tensor_scalar(
                                cand_i[:, 8 * qt:8 * qt + 8], i8f[:],
                                float(2500 * qt), None, op0=ALU.add)
                            nscan[0] += 1

                    for grp in range(5):
                        gp3 = pbig()
                        for tt in range(4):
                            n0 = (grp * 4 + tt) * TW
                            o = gp3[:, tt, 0:TW]
                            for k in range(2):
                                yhk = y2h[:, k, bsl]
                                ylk = y2l[:, k, bsl]
                                wh = fc3h[:, k, n0:n0 + TW]
                                wl = fc3l[:, k, n0:n0 + TW]
                                nc.tensor.matmul(o, yhk, wh, start=(k == 0),
                                                 stop=False)
                                nc.tensor.matmul(o, yhk, wl, start=False, stop=False)
                                nc.tensor.matmul(o, ylk, wh, start=False, stop=False)
                                nc.tensor.matmul(o, ylk, wl, start=False,
                                                 stop=(k == 1))
                        # evacuate per psum tile (+bias) into quarter tiles
                        for tt in range(4):
                            n0 = (grp * 4 + tt) * TW
                            qt = n0 // 2500
                            nc.vector.tensor_add(
                                lq[qt][:, n0 - 2500 * qt:n0 - 2500 * qt + TW],
                                gp3[:, tt, 0:TW],
                                b3rep[:, n0:n0 + TW])
                        if dbg and t == 0 and bc == 0:
                            for tt in range(4):
                                n0 = (grp * 4 + tt) * TW
                                qt = n0 // 2500
                                nc.sync.dma_start(
                                    dbg_log[:, n0:n0 + TW],
                                    lq[qt][:, n0 - 2500 * qt:n0 - 2500 * qt + TW])
                        scan_ready(grp * 2000 + 2000)

                    # merge 32 candidates
                    vm8 = wk.tile([128, 8], F32, tag="vm8", name="vm8")
                    pm8 = wk.tile([128, 8], U32, tag="pm8", name="pm8")
                    nc.vector.max(vm8[:], cand_v[:])
                    nc.vector.max_index(pm8[:], vm8[:], cand_v[:])
                    pmf = wk.tile([128, 8], F32, tag="pmf", name="pmf")
                    nc.vector.tensor_copy(pmf[:], pm8[:])
                    nk = 4 if t == 0 else 1
                    qsel = wk.tile([128, 4], F32, tag="qsel", name="qsel")
                    for kk in range(nk):
                        ohp = wk.tile([128, 32], F32, tag="ohp", name="ohp")
                        nc.vector.tensor_scalar(ohp[:], io32[:], pmf[:, kk:kk + 1],
                                                None, op0=ALU.is_equal)
                        tmq = wk.tile([128, 32], F32, tag="tmq", name="tmq")
                        nc.vector.tensor_mul(tmq[:], ohp[:], cand_i[:])
                        nc.vector.tensor_reduce(qsel[:, kk:kk + 1], tmq[:],
                                                axis=mybir.AxisListType.X,
                                                op=ALU.add)
                    if t == 0:
                        nc.vector.tensor_copy(outi[:, bc, 0:4], qsel[:, 0:4])
                    else:
                        nc.vector.tensor_copy(outi[:, bc, 4 + t - 1:5 + t - 1],
                                              qsel[:, 0:1])
                    if t == delta - 1:
                        continue
                    qf = wk.tile([128, 1], F32, tag="qf", name="qf")
                    nc.vector.tensor_copy(qf[:], qsel[:, 0:1])
                    # ohw[b,j] = (100j <= q) & (100j > q-100)
                    m_ge = wk.tile([128, 100], F32, tag="mge", name="mge", bufs=1)
                    nc.vector.tensor_scalar(m_ge[:], io100[:], qf[:], None,
                                            op0=ALU.is_le)
                    qm = wk.tile([128, 1], F32, tag="qm", name="qm")
                    nc.vector.tensor_scalar(qm[:], qf[:], -100.0, None, op0=ALU.add)
                    m_lt = wk.tile([128, 100], F32, tag="mlt", name="mlt", bufs=1)
                    nc.vector.tensor_scalar(m_lt[:], io100[:], qm[:], None,
                                            op0=ALU.is_gt)
                    ohw = wk.tile([128, 100], F32, tag="ohw", name="ohw", bufs=1)
                    nc.vector.tensor_mul(ohw[:], m_ge[:], m_lt[:])
                    tm = wk.tile([128, 100], F32, tag="tm", name="tm", bufs=1)
                    nc.vector.tensor_mul(tm[:], ohw[:], io_f[:])
                    fwf = wk.tile([128, 1], F32, tag="fwf", name="fwf")
                    nc.vector.tensor_reduce(fwf[:], tm[:], axis=mybir.AxisListType.X,
                                            op=ALU.add)
                    flf = wk.tile([128, 1], F32, tag="flf", name="flf")
                    nc.vector.tensor_scalar(flf[:], fwf[:], -100.0, qf[:],
                                            op0=ALU.mult, op1=ALU.add)
                    ohl = wk.tile([128, 100], F32, tag="ohl", name="ohl", bufs=1)
                    nc.vector.tensor_scalar(ohl[:], io_f[:], flf[:], None,
                                            op0=ALU.is_equal)
                    ptr = pbig()
                    pw = ptr[0:100, 0, 0:128]
                    nc.tensor.transpose(pw, ohw[:], ident[:])
                    nc.vector.tensor_copy(ohwT[:, bsl128(bc)], pw)
                    pl = ptr[0:100, 1, 0:128]
                    nc.tensor.transpose(pl, ohl[:], ident[:])
                    nc.vector.tensor_copy(ohlT[:, bsl128(bc)], pl)

                if t == delta - 1:
                    continue
                # embedding gather matmuls + bias
                pet = pbig()
                pe0 = pet[:, 0, 0:BS]
                pe1 = pet[:, 1, 0:BS]
                nc.tensor.matmul(pe0, fcqw[:], ohwT[:], start=True, stop=True)
                nc.tensor.matmul(pe1, fcql[:], ohlT[:], start=True, stop=True)
                nc.scalar.activation(emb_t[:, 0, :], pe0, AF.Identity,
                                     bias=fcqwb[:])
                nc.scalar.activation(emb_t[:, 1, :], pe1, AF.Identity,
                                     bias=fcqlb[:])

            for bc in range(2):
                nc.sync.dma_start(idx_out[bc], outi[:, bc, :])
    nc.finalize()
    return nc


def bsl128(bc):
    return slice(128 * bc, 128 * (bc + 1))


def _prep_shared(inputs):
    f32 = np.float32
    bf = ml_dtypes.bfloat16

    def fm(w):  # [out,in] -> lhsT layout [2,128,out]
        wt = np.ascontiguousarray(w.T.astype(f32))        # [in, out]
        return wt.reshape(2, 128, wt.shape[1])

    fc3T = np.ascontiguousarray(inputs["fc3_W"].T.astype(f32))  # [256, 10000]
    fc3h = fc3T.astype(bf)
    fc3l = (fc3T - fc3h.astype(f32)).astype(bf)

    shared = {
        "w1ihT": fm(inputs["lstm1_Wih"]),
        "w1hhT": fm(inputs["lstm1_Whh"]),
        "w2ihT": fm(inputs["lstm2_Wih"]),
        "w2hhT": fm(inputs["lstm2_Whh"]),
        "fc1T": fm(inputs["fc1_W"]),
        "fc2T": fm(inputs["fc2_W"]),
        "fc3Th": fc3h.reshape(2, 128, Q),
        "fc3Tl": fc3l.reshape(2, 128, Q),
        "fcqwT": np.ascontiguousarray(inputs["fcqw_W"].T.astype(f32))[:, :],
        "fcqlT": np.ascontiguousarray(inputs["fcql_W"].T.astype(f32))[:, :],
        "b1r": inputs["lstm1_b"].astype(f32).reshape(8, 128).T.copy(),
        "b2r": inputs["lstm2_b"].astype(f32).reshape(8, 128).T.copy(),
        "fc1br": inputs["fc1_b"].astype(f32).reshape(2, 128).T.copy(),
        "fc2br": inputs["fc2_b"].astype(f32).reshape(2, 128).T.copy(),
        "b3rep": np.ascontiguousarray(
            np.broadcast_to(inputs["fc3_b"].astype(f32), (128, Q))),
        "fcqwb": inputs["fcqw_b"].astype(f32).reshape(128, 1),
        "fcqlb": inputs["fcql_b"].astype(f32).reshape(128, 1),
    }
    return shared


def _per_core(inputs, c):
    f32 = np.float32
    sl = slice(c * BS, (c + 1) * BS)

    def fmT(a):  # [BS, 256] -> [2, 128, BS]
        return np.ascontiguousarray(a.T.astype(f32)).reshape(2, 128, BS)

    return {
        "x_fm": fmT(inputs["x"][sl, 0, :]),
        "h1_fm": fmT(inputs["h1"][0, sl]),
        "c1_fm": fmT(inputs["c1"][0, sl]),
        "h2_fm": fmT(inputs["h2"][0, sl]),
        "c2_fm": fmT(inputs["c2"][0, sl]),
    }


def kernel(**inputs):
    key = "nc"
    if key not in _CACHE:
        _CACHE[key] = _build_nc()
    nc = _CACHE[key]

    shared = _prep_shared(inputs)
    in_maps = []
    for c in range(NCORES):
        m = dict(shared)
        m.update(_per_core(inputs, c))
        in_maps.append(m)

    from concourse.bass_utils import run_bass_kernel_spmd
    res = run_bass_kernel_spmd(nc, in_maps, list(range(NCORES)))
    return assemble(res.results)


def assemble(results):
    traj = np.zeros((B, DELTA, K4, 2), np.float32)
    for c, r in enumerate(results):
        idx = r["idx_out"].reshape(2, 128, 20).astype(np.int64)
        for bc in range(2):
            rows = slice(c * BS + bc * 128, c * BS + (bc + 1) * 128)
            top4 = idx[bc, :, 0:4]
            traj[rows, 0, :, 0] = (top4 % QL).astype(np.float32)
            traj[rows, 0, :, 1] = (top4 // QL).astype(np.float32)
            greedy = idx[bc, :, 4:4 + DELTA - 1]
            traj[rows, 1:, 0, 0] = (greedy % QL).astype(np.float32)
            traj[rows, 1:, 0, 1] = (greedy // QL).astype(np.float32)
    return traj



# revision 24
# speedup vs baseline: 1.3698x; 1.3698x over previous
"""Trainium2 Bass kernel for nn_Decoder (2-layer LSTM + 3 FC + top-k decode).

Strategy: pure data parallelism over batch (2048 -> 8 cores x 256).
Feature-major activations [feat, batch]. All matmuls are 3-term bf16
splits (hi/lo), which is empirically exact for every argmax decision.
fc1/fc2/fc3 are folded on the host into one 256->10000 matmul (fp64
compose). For steps >= 1 the LSTM1 input matmul is replaced by one-hot
table matmuls (tables = W1ih @ fcq{w,l}_W, host fp64). The decode
pipeline is split into two 128-row chunks so PE work (gates, fc3)
overlaps the DVE scan/merge/one-hot of the other chunk.
"""
import numpy as np
import ml_dtypes

B, D, H = 2048, 256, 256
K4, QW, QL, DELTA = 4, 100, 100, 16
Q = QW * QL
NCORES = 8
BS = B // NCORES          # 256 rows per core
TW = 500                  # fc3 tile width
NT = Q // TW              # 20 tiles per chunk
G4 = 4 * H                # 1024 gates

_CACHE = {}


def _build_nc(delta=DELTA):
    import concourse.mybir as mybir
    import concourse.tile as tile
    import concourse.bacc as bacc
    from concourse.masks import make_identity

    F32 = mybir.dt.float32
    BF16 = mybir.dt.bfloat16
    U32 = mybir.dt.uint32
    AF = mybir.ActivationFunctionType
    ALU = mybir.AluOpType

    nc = bacc.Bacc(None, target_bir_lowering=False, debug=False)

    def din(name, shape, dt=F32):
        return nc.dram_tensor(name, shape, dt, kind="ExternalInput")

    # per-core inputs
    xh_in = din("xh", [2, 128, BS], BF16)
    xl_in = din("xl", [2, 128, BS], BF16)
    c1_in = din("c1_fm", [2, 128, BS])
    c2_in = din("c2_fm", [2, 128, BS])
    h1h_in = din("h1h", [2, 128, BS], BF16)
    h1l_in = din("h1l", [2, 128, BS], BF16)
    h2h_in = din("h2h", [2, 128, BS], BF16)
    h2l_in = din("h2l", [2, 128, BS], BF16)
    # shared weights (bf16 hi/lo pairs, lhsT layout)
    w1ihh_in = din("w1ihTh", [2, 128, G4], BF16)
    w1ihl_in = din("w1ihTl", [2, 128, G4], BF16)
    w1hhh_in = din("w1hhTh", [2, 128, G4], BF16)
    w1hhl_in = din("w1hhTl", [2, 128, G4], BF16)
    w2ihh_in = din("w2ihTh", [2, 128, G4], BF16)
    w2ihl_in = din("w2ihTl", [2, 128, G4], BF16)
    w2hhh_in = din("w2hhTh", [2, 128, G4], BF16)
    w2hhl_in = din("w2hhTl", [2, 128, G4], BF16)
    w3h_in = din("w3Th", [2, 128, Q], BF16)
    w3l_in = din("w3Tl", [2, 128, Q], BF16)
    awh_in = din("awTh", [100, G4], BF16)
    awl_in = din("awTl", [100, G4], BF16)
    alh_in = din("alTh", [100, G4], BF16)
    all_in = din("alTl", [100, G4], BF16)
    b1r_in = din("b1r", [128, 8])
    b1rf_in = din("b1rf", [128, 8])
    b2r_in = din("b2r", [128, 8])
    b3t_in = din("b3t", [3, Q], mybir.dt.bfloat16)

    idx_out = nc.dram_tensor("idx_out", [2, 128, 20], U32, kind="ExternalOutput")

    with tile.TileContext(nc) as tc:
        with (
            tc.tile_pool(name="wp", bufs=1) as wp,
            tc.tile_pool(name="st", bufs=1) as st,
            tc.tile_pool(name="wk", bufs=2) as wk,
            tc.tile_pool(name="p3", bufs=3, space="PSUM") as p3,
            tc.tile_pool(name="pg", bufs=2, space="PSUM") as pg,
        ):
            # ---- weight / const loads (ordered by first use) ----
            def wload(src, shape, tag, dt=F32):
                t = wp.tile(shape, dt, tag=tag, name=tag)
                if len(shape) == 3 and shape[1] == 2:
                    nc.sync.dma_start(t[:], src[:].rearrange("c p f -> p c f"))
                else:
                    nc.sync.dma_start(t[:], src[:])
                return t

            w1ihh = wload(w1ihh_in, [128, 2, G4], "w1ihh", BF16)
            w1ihl = wload(w1ihl_in, [128, 2, G4], "w1ihl", BF16)
            w1hhh = wload(w1hhh_in, [128, 2, G4], "w1hhh", BF16)
            w1hhl = wload(w1hhl_in, [128, 2, G4], "w1hhl", BF16)
            b1r = wload(b1r_in, [128, 8], "b1r")
            b2r = wload(b2r_in, [128, 8], "b2r")
            w2ihh = wload(w2ihh_in, [128, 2, G4], "w2ihh", BF16)
            w2ihl = wload(w2ihl_in, [128, 2, G4], "w2ihl", BF16)
            w2hhh = wload(w2hhh_in, [128, 2, G4], "w2hhh", BF16)
            w2hhl = wload(w2hhl_in, [128, 2, G4], "w2hhl", BF16)
            w3h = wload(w3h_in, [128, 2, Q], "w3h", BF16)
            w3l = wload(w3l_in, [128, 2, Q], "w3l", BF16)
            b3t = wload(b3t_in, [3, Q], "b3t", BF16)
            one3 = wp.tile([3, 128], BF16)
            nc.vector.memset(one3[:], 1.0)
            awh = wload(awh_in, [100, G4], "awh", BF16)
            awl = wload(awl_in, [100, G4], "awl", BF16)
            alh = wload(alh_in, [100, G4], "alh", BF16)
            all_ = wload(all_in, [100, G4], "all", BF16)
            b1rf = wload(b1rf_in, [128, 8], "b1rf")

            ident = wp.tile([128, 128], F32)
            make_identity(nc, ident[:])
            io_f = wp.tile([128, 100], F32)
            nc.gpsimd.iota(io_f[:], pattern=[[1, 100]], base=0,
                           channel_multiplier=0,
                           allow_small_or_imprecise_dtypes=True)
            io100 = wp.tile([128, 100], F32)
            nc.gpsimd.iota(io100[:], pattern=[[100, 100]], base=0,
                           channel_multiplier=0,
                           allow_small_or_imprecise_dtypes=True)
            io40 = wp.tile([128, 40], F32)
            nc.gpsimd.iota(io40[:], pattern=[[1, 40]], base=0,
                           channel_multiplier=0,
                           allow_small_or_imprecise_dtypes=True)

            # ---- persistent state ----
            def sload(src, tag, dt=F32):
                t = st.tile([128, 2, BS], dt, tag=tag, name=tag)
                nc.sync.dma_start(t[:], src[:].rearrange("c p b -> p c b"))
                return t

            xh = sload(xh_in, "xh", BF16)
            xl = sload(xl_in, "xl", BF16)
            c1_t = sload(c1_in, "c1")
            c2_t = sload(c2_in, "c2")
            h1h = sload(h1h_in, "h1h", BF16)
            h1l = sload(h1l_in, "h1l", BF16)
            h2h = sload(h2h_in, "h2h", BF16)
            h2l = sload(h2l_in, "h2l", BF16)
            h1_t = st.tile([128, 2, BS], F32, tag="h1", name="h1")
            h2_t = st.tile([128, 2, BS], F32, tag="h2", name="h2")
            ohwT = st.tile([100, BS], BF16, tag="ohwT", name="ohwT")
            ohlT = st.tile([100, BS], BF16, tag="ohlT", name="ohlT")
            outi = st.tile([128, 2, 20], U32, tag="outi", name="outi")
            nc.vector.memset(outi[:], 0)

            def bsl(bc):
                return slice(128 * bc, 128 * (bc + 1))

            # ---- per-chunk LSTM layer ----
            def gates_layer1(bc, t):
                """gates1 psum for chunk bc: tables/x part + whh1."""
                gp = pg.tile([128, 8, 128], F32, tag="g1", name="g1")
                bs = bsl(bc)
                for g in range(8):
                    sl = slice(128 * g, 128 * (g + 1))
                    o = gp[:, g, :]
                    # recurrent part (3-term bf16)
                    for k in range(2):
                        nc.tensor.matmul(o, w1hhh[:, k, sl], h1h[:, k, bs],
                                         start=(k == 0), stop=False)
                        nc.tensor.matmul(o, w1hhh[:, k, sl], h1l[:, k, bs],
                                         start=False, stop=False)
                        nc.tensor.matmul(o, w1hhl[:, k, sl], h1h[:, k, bs],
                                         start=False, stop=False)
                    if t == 0:
                        for k in range(2):
                            nc.tensor.matmul(o, w1ihh[:, k, sl], xh[:, k, bs],
                                             start=False, stop=False)
                            nc.tensor.matmul(o, w1ihh[:, k, sl], xl[:, k, bs],
                                             start=False, stop=False)
                            nc.tensor.matmul(o, w1ihl[:, k, sl], xh[:, k, bs],
                                             start=False, stop=(k == 1))
                    else:
                        nc.tensor.matmul(o, awh[:, sl], ohwT[:, bs],
                                         start=False, stop=False)
                        nc.tensor.matmul(o, awl[:, sl], ohwT[:, bs],
                                         start=False, stop=False)
                        nc.tensor.matmul(o, alh[:, sl], ohlT[:, bs],
                                         start=False, stop=False)
                        nc.tensor.matmul(o, all_[:, sl], ohlT[:, bs],
                                         start=False, stop=True)
                return gp

            def gates_layer2(bc):
                gp = pg.tile([128, 8, 128], F32, tag="g1", name="g2")
                bs = bsl(bc)
                for g in range(8):
                    sl = slice(128 * g, 128 * (g + 1))
                    o = gp[:, g, :]
                    for k in range(2):
                        nc.tensor.matmul(o, w2ihh[:, k, sl], h1h[:, k, bs],
                                         start=(k == 0), stop=False)
                        nc.tensor.matmul(o, w2ihh[:, k, sl], h1l[:, k, bs],
                                         start=False, stop=False)
                        nc.tensor.matmul(o, w2ihl[:, k, sl], h1h[:, k, bs],
                                         start=False, stop=False)
                    for k in range(2):
                        nc.tensor.matmul(o, w2hhh[:, k, sl], h2h[:, k, bs],
                                         start=False, stop=False)
                        nc.tensor.matmul(o, w2hhh[:, k, sl], h2l[:, k, bs],
                                         start=False, stop=False)
                        nc.tensor.matmul(o, w2hhl[:, k, sl], h2h[:, k, bs],
                                         start=False, stop=(k == 1))
                return gp

            def pointwise(bc, gp, cT, hT, hh, hl, br):
                """activations + LSTM cell update + bf16 split, chunk bc."""
                bs = bsl(bc)
                si = wk.tile([128, 2, 128], F32, tag="si")
                sf = wk.tile([128, 2, 128], F32, tag="sf")
                tg = wk.tile([128, 2, 128], F32, tag="tg")
                so = wk.tile([128, 2, 128], F32, tag="so")
                for ch in range(2):
                    nc.scalar.activation(si[:, ch, :], gp[:, 0 + ch, :],
                                         AF.Sigmoid, bias=br[:, 0 + ch:1 + ch])
                    nc.scalar.activation(sf[:, ch, :], gp[:, 2 + ch, :],
                                         AF.Sigmoid, bias=br[:, 2 + ch:3 + ch])
                    nc.scalar.activation(tg[:, ch, :], gp[:, 4 + ch, :],
                                         AF.Tanh, bias=br[:, 4 + ch:5 + ch])
                    nc.scalar.activation(so[:, ch, :], gp[:, 6 + ch, :],
                                         AF.Sigmoid, bias=br[:, 6 + ch:7 + ch])
                csl = cT[:, :, bs]
                hsl = hT[:, :, bs]
                t1 = wk.tile([128, 2, 128], F32, tag="t1", bufs=1)
                t2 = wk.tile([128, 2, 128], F32, tag="t2", bufs=1)
                nc.vector.tensor_mul(t1[:], sf[:], csl)
                nc.vector.tensor_mul(t2[:], si[:], tg[:])
                nc.vector.tensor_add(csl, t1[:], t2[:])
                t3 = wk.tile([128, 2, 128], F32, tag="t3", bufs=1)
                nc.scalar.activation(t3[:], csl, AF.Tanh)
                nc.vector.tensor_mul(hsl, so[:], t3[:])
                nc.vector.tensor_copy(hh[:, :, bs], hsl)
                nc.vector.tensor_sub(hl[:, :, bs], hsl, hh[:, :, bs])

            # ---- fc3 + scan for one chunk ----
            def fc3_scan(bc):
                bs = bsl(bc)
                lq = [wk.tile([128, 2000], F32, tag="logq", name="logq",
                              bufs=2) for _ in range(5)]
                cand_v = wk.tile([128, 40], F32, tag="candv", name="candv")
                cand_i = wk.tile([128, 40], F32, tag="candi", name="candi")
                stats = [(h2h[:, 0, bs], w3h[:, 0, :]),
                         (h2h[:, 1, bs], w3h[:, 1, :]),
                         (h2l[:, 0, bs], w3h[:, 0, :]),
                         (h2l[:, 1, bs], w3h[:, 1, :]),
                         (h2h[:, 0, bs], w3l[:, 0, :]),
                         (h2h[:, 1, bs], w3l[:, 1, :])]
                for tt in range(NT):
                    n0 = tt * TW
                    pt = p3.tile([128, 512], F32, tag="fc3p", name="fc3p")
                    o = pt[:, 0:TW]
                    # seed PSUM with the (3-term bf16) fc3 bias, then accumulate
                    nc.tensor.matmul(o, one3[:], b3t[:, n0:n0 + TW],
                                     start=True, stop=False)
                    for j, (stat, w) in enumerate(stats):
                        nc.tensor.matmul(o, stat, w[:, n0:n0 + TW],
                                         start=False, stop=(j == 5))
                    qt = n0 // 2000
                    off = n0 - 2000 * qt
                    nc.scalar.copy(lq[qt][:, off:off + TW], o)
                    if (tt + 1) % 4 == 0:
                        m8q = wk.tile([128, 8], F32, tag="m8q", name="m8q")
                        i8q = wk.tile([128, 8], U32, tag="i8q", name="i8q")
                        nc.vector.max(m8q[:], lq[qt][:])
                        nc.vector.max_index(i8q[:], m8q[:], lq[qt][:])
                        nc.vector.tensor_copy(cand_v[:, 8 * qt:8 * qt + 8],
                                              m8q[:])
                        i8f = wk.tile([128, 8], F32, tag="i8f", name="i8f")
                        nc.vector.tensor_copy(i8f[:], i8q[:])
                        nc.vector.tensor_scalar(
                            cand_i[:, 8 * qt:8 * qt + 8], i8f[:],
                            float(2000 * qt), None, op0=ALU.add)
                return cand_v, cand_i

            def merge_onehot(bc, t, cand_v, cand_i):
                """top-k merge, trajectory index write, one-hot build."""
                vm8 = wk.tile([128, 8], F32, tag="vm8", name="vm8")
                pm8 = wk.tile([128, 8], U32, tag="pm8", name="pm8")
                nc.vector.max(vm8[:], cand_v[:])
                nc.vector.max_index(pm8[:], vm8[:], cand_v[:])
                pmf = wk.tile([128, 8], F32, tag="pmf", name="pmf")
                nc.vector.tensor_copy(pmf[:], pm8[:])
                nk = 4 if t == 0 else 1
                qsel = wk.tile([128, 4], F32, tag="qsel", name="qsel")
                for kk in range(nk):
                    ohp = wk.tile([128, 40], F32, tag="ohp", name="ohp")
                    nc.vector.tensor_scalar(ohp[:], io40[:], pmf[:, kk:kk + 1],
                                            None, op0=ALU.is_equal)
                    tmq = wk.tile([128, 40], F32, tag="tmq", name="tmq")
                    nc.vector.tensor_mul(tmq[:], ohp[:], cand_i[:])
                    nc.vector.tensor_reduce(qsel[:, kk:kk + 1], tmq[:],
                                            axis=mybir.AxisListType.X,
                                            op=ALU.add)
                if t == 0:
                    nc.vector.tensor_copy(outi[:, bc, 0:4], qsel[:, 0:4])
                else:
                    nc.vector.tensor_copy(outi[:, bc, 4 + t - 1:5 + t - 1],
                                          qsel[:, 0:1])
                if t == delta - 1:
                    return None, None
                qf = wk.tile([128, 1], F32, tag="qf", name="qf")
                nc.vector.tensor_copy(qf[:], qsel[:, 0:1])
                m_ge = wk.tile([128, 100], F32, tag="mge", name="mge", bufs=1)
                nc.vector.tensor_scalar(m_ge[:], io100[:], qf[:], None,
                                        op0=ALU.is_le)
                qm = wk.tile([128, 1], F32, tag="qm", name="qm")
                nc.vector.tensor_scalar(qm[:], qf[:], -100.0, None, op0=ALU.add)
                m_lt = wk.tile([128, 100], F32, tag="mlt", name="mlt", bufs=1)
                nc.vector.tensor_scalar(m_lt[:], io100[:], qm[:], None,
                                        op0=ALU.is_gt)
                ohw = wk.tile([128, 100], F32, tag="ohw", name="ohw", bufs=2)
                nc.vector.tensor_mul(ohw[:], m_ge[:], m_lt[:])
                tm = wk.tile([128, 100], F32, tag="tm", name="tm", bufs=1)
                nc.vector.tensor_mul(tm[:], ohw[:], io_f[:])
                fwf = wk.tile([128, 1], F32, tag="fwf", name="fwf")
                nc.vector.tensor_reduce(fwf[:], tm[:], axis=mybir.AxisListType.X,
                                        op=ALU.add)
                flf = wk.tile([128, 1], F32, tag="flf", name="flf")
                nc.vector.tensor_scalar(flf[:], fwf[:], -100.0, qf[:],
                                        op0=ALU.mult, op1=ALU.add)
                ohl = wk.tile([128, 100], F32, tag="ohl", name="ohl", bufs=2)
                nc.vector.tensor_scalar(ohl[:], io_f[:], flf[:], None,
                                        op0=ALU.is_equal)
                return ohw, ohl

            def trans_oh(bc, ohw, ohl):
                """transpose one-hots into [100, BS] bf16 table operands."""
                bs = bsl(bc)
                pw = p3.tile([128, 512], F32, tag="fc3p", name="ptw")
                nc.tensor.transpose(pw[0:100, 0:128], ohw[:], ident[:])
                nc.vector.tensor_copy(ohwT[:, bs], pw[0:100, 0:128])
                pl = p3.tile([128, 512], F32, tag="fc3p", name="ptl")
                nc.tensor.transpose(pl[0:100, 0:128], ohl[:], ident[:])
                nc.vector.tensor_copy(ohlT[:, bs], pl[0:100, 0:128])

            # ================= main loop =================
            for t in range(delta):
                # LSTM phase (consumes ohwT/ohlT from step t-1)
                for bc in range(2):
                    gp1 = gates_layer1(bc, t)
                    pointwise(bc, gp1, c1_t, h1_t, h1h, h1l,
                              b1r if t == 0 else b1rf)
                for bc in range(2):
                    gp2 = gates_layer2(bc)
                    pointwise(bc, gp2, c2_t, h2_t, h2h, h2l, b2r)
                # fc3 + scan + merge + one-hot, chunk-pipelined
                cv0, ci0 = fc3_scan(0)
                m0 = merge_onehot(0, t, cv0, ci0)
                cv1, ci1 = fc3_scan(1)
                if m0[0] is not None:
                    trans_oh(0, m0[0], m0[1])
                m1 = merge_onehot(1, t, cv1, ci1)
                if m1[0] is not None:
                    trans_oh(1, m1[0], m1[1])

            for bc in range(2):
                nc.sync.dma_start(idx_out[bc], outi[:, bc, :])
    nc.finalize()
    return nc


def _prep_shared(inputs):
    f32, f64 = np.float32, np.float64
    bf = ml_dtypes.bfloat16

    def split(a):
        ah = a.astype(bf)
        al = (a.astype(f32) - ah.astype(f32)).astype(bf)
        return ah, al

    def fmT(w):  # [out, in] -> lhsT chunks [2, 128, out]
        wt = np.ascontiguousarray(w.T.astype(f32))
        return wt.reshape(2, 128, wt.shape[1])

    W = {k: np.asarray(v) for k, v in inputs.items()}
    fc1, fc2, fc3 = (W['fc1_W'].astype(f64), W['fc2_W'].astype(f64),
                     W['fc3_W'].astype(f64))
    W3f = (fc3 @ fc2 @ fc1).astype(f32)                       # [Q, 256]
    b3f = (W['fc3_b'].astype(f64) + fc3 @ W['fc2_b'].astype(f64)
           + (fc3 @ fc2) @ W['fc1_b'].astype(f64)).astype(f32)
    Aw = (W['lstm1_Wih'][:, :128].astype(f64)
          @ W['fcqw_W'].astype(f64)).astype(f32)              # [1024, 100]
    Al = (W['lstm1_Wih'][:, 128:].astype(f64)
          @ W['fcql_W'].astype(f64)).astype(f32)
    embb = np.concatenate([W['fcqw_b'], W['fcql_b']]).astype(f64)
    b1f = (W['lstm1_b'].astype(f64)
           + W['lstm1_Wih'].astype(f64) @ embb).astype(f32)

    shared = {}
    for name, w in (("w1ih", W['lstm1_Wih']), ("w1hh", W['lstm1_Whh']),
                    ("w2ih", W['lstm2_Wih']), ("w2hh", W['lstm2_Whh'])):
        h_, l_ = split(fmT(w))
        shared[name + "Th"] = h_
        shared[name + "Tl"] = l_
    shared["w3Th"], shared["w3Tl"] = split(fmT(W3f))
    shared["awTh"], shared["awTl"] = split(np.ascontiguousarray(Aw.T))
    shared["alTh"], shared["alTl"] = split(np.ascontiguousarray(Al.T))
    shared["b1r"] = W['lstm1_b'].astype(f32).reshape(8, 128).T.copy()
    shared["b1rf"] = b1f.reshape(8, 128).T.copy()
    shared["b2r"] = W['lstm2_b'].astype(f32).reshape(8, 128).T.copy()
    # fc3 bias as 3 bf16 terms (seeded into PSUM via a K=3 ones matmul)
    b3a = b3f.astype(bf)
    r1 = (b3f - b3a.astype(f32)).astype(f32)
    b3b = r1.astype(bf)
    b3c = (r1 - b3b.astype(f32)).astype(bf)
    shared["b3t"] = np.ascontiguousarray(np.stack([b3a, b3b, b3c]))
    return shared


def _per_core(inputs, c):
    f32 = np.float32
    bf = ml_dtypes.bfloat16
    sl = slice(c * BS, (c + 1) * BS)

    def fmT(a):  # [BS, 256] -> [2, 128, BS]
        return np.ascontiguousarray(a.T.astype(f32)).reshape(2, 128, BS)

    def split(a):
        ah = a.astype(bf)
        al = (a - ah.astype(f32)).astype(bf)
        return ah, al

    x = fmT(np.asarray(inputs["x"])[sl, 0, :])
    h1 = fmT(np.asarray(inputs["h1"])[0, sl])
    h2 = fmT(np.asarray(inputs["h2"])[0, sl])
    xh, xl = split(x)
    h1h, h1l = split(h1)
    h2h, h2l = split(h2)
    return {
        "xh": xh, "xl": xl,
        "c1_fm": fmT(np.asarray(inputs["c1"])[0, sl]),
        "c2_fm": fmT(np.asarray(inputs["c2"])[0, sl]),
        "h1h": h1h, "h1l": h1l, "h2h": h2h, "h2l": h2l,
    }


def kernel(**inputs):
    key = "nc"
    if key not in _CACHE:
        _CACHE[key] = _build_nc()
    nc = _CACHE[key]

    shared = _prep_shared(inputs)
    in_maps = []
    for c in range(NCORES):
        m = dict(shared)
        m.update(_per_core(inputs, c))
        in_maps.append(m)

    from concourse.bass_utils import run_bass_kernel_spmd
    res = run_bass_kernel_spmd(nc, in_maps, list(range(NCORES)))
    return assemble(res.results)


def assemble(results):
    traj = np.zeros((B, DELTA, K4, 2), np.float32)
    for c, r in enumerate(results):
        idx = r["idx_out"].reshape(2, 128, 20).astype(np.int64)
        for bc in range(2):
            rows = slice(c * BS + bc * 128, c * BS + (bc + 1) * 128)
            top4 = idx[bc, :, 0:4]
            traj[rows, 0, :, 0] = (top4 % QL).astype(np.float32)
            traj[rows, 0, :, 1] = (top4 // QL).astype(np.float32)
            greedy = idx[bc, :, 4:4 + DELTA - 1]
            traj[rows, 1:, 0, 0] = (greedy % QL).astype(np.float32)
            traj[rows, 1:, 0, 1] = (greedy // QL).astype(np.float32)
    return traj
